# revision 30
# baseline (speedup 1.0000x reference)
"""DigitCaps dynamic-routing kernel for Trainium2 (8 NeuronCores, Bass/Tile).

Math (per routing iteration, reformulated to avoid materializing u_hat):
    u_hat[b,i,j,u] = sum_k W[i,j,u,k] * x[b,k,i]
    s[b,ju]  = sum_{ki} X[ki,b] * (c[i,j] * W[ki,ju])          (PE matmul, K=9216)
    v        = squash(s)  with the reference's quirky j-axis norm
    G[ki,ju] = sum_b X[b,ki] * v[b,ju]                         (PE matmul, K=64)
    b[i,j]   = sum_{k,u} W[ki,ju] * G[ki,ju]                   (DVE product+reduce)
    b is summed over the 8 cores each routing iteration (batch mean).

Sharding: data-parallel over batch B=512 -> 64 rows per core; W replicated.

Design notes (v3, informed by perfetto/NTFF traces of the 180us v1/183us v2):
  - the ncfw collective stack runs a ~35-43us cold barrier (core-launch skew)
    that starts at ~21us regardless of kernel instructions, plus a fixed
    ~11us first-collective wakeup after it.  it0 (uniform c, no exp) plus
    all input DMA and the PE transposes execute inside that window, so
    it0's collective fires the moment the barrier clears.  A separate
    warm-up collective does NOT help: the wakeup+cold cost is paid
    serially after the barrier either way (measured both ways).
  - the cross-core b exchange is an AllGather + local DVE fold sum, not an
    AllReduce: the 8-core mesh AR floor is ~10-24us while AG is ~5-9us,
    and the <=23KB payload makes everything floor-dominated.  Each core
    stages its [128, nt*J] half with one plain DMA, the AG stacks ranks on
    the row axis, two plain DMAs bring back [128, 8, nt*J], and 3 bf16
    fold adds reproduce the sum.  No DMA-transpose anywhere: Tile
    serializes those against all other DMA traffic (incl. collectives).
  - collective doorbells pay ~2-6us of DMA-completion-semaphore latency
    after the staging write lands in HBM; the it1 exchange is split 5/4 so
    half A's doorbell fires mid b-update, and the smaller half B keeps the
    last flight and it2's trailing wp/s block short.
  - wp = exp(b_sum/B) * W runs with the exp+broadcast fused on the ACT
    engine into a materialized eU4 (k-half, u replica), so the two wp
    tensor_muls per chunk are fully contiguous bf16 -> DVE 2x mode
    (measured 488ns vs 1490ns for the v1 broadcast-AP form).
  - b-update DVE work is processed two i-chunks per instruction (one
    product + 3 fold adds + one reduce per pair), amortizing the ~60ns
    DVE instruction overheads; PSUM keeps the 4-bank ping-pong.
  - both x layouts come from the host: +1.18MB of DMA replaces 72 PE
    transposes + 72 ACT evacuations (~9us PE + ~18us ACT in it0), pulling
    the it0 doorbell from ~65us to ~55us, under the barrier end.
  - input DMA uses all three DMA rings (sync: x1+x2, scalar+gpsimd: W
    halves) with only 6 HWDGE pieces, staying inside the 8 Tile HWDGE
    completion-sem lanes (more pieces stall ~10us on lane recycling).
  - the PE HAM clock-gate idles at 1.2GHz until ~3.4us of sustained
    activity and refreezes after ~3.4us idle; junk-transpose chains warm
    it at kernel start and, gated on each exchange result, during each
    exchange's result ramp so the s-matmuls run at 2.4GHz.
  - everything on the PE runs bf16 (fp32 LOW_HIGH matmuls are ~5x slower);
    measured end-to-end L2 err ~4e-3 vs the 2e-2 gate.
  - squash sqrt is a DVE Newton rsqrt (quake seed + 1 iteration) so the
    ACT exp table never reloads (a table switch costs ~1.5us serial).
"""

import sys

sys.path.insert(0, "/opt/trn_rl_repo")

from contextlib import ExitStack

import numpy as np

B = 512
NCORES = 8
BL = B // NCORES  # 64 local batch rows
K = 8             # in_units (primary capsule dim)
IC = 1152         # in_channels (number of primary capsules)
J = 10            # num_units (output capsules)
U = 16            # unit_size
JU = J * U        # 160
NT = IC // 128    # 9 i-chunks of 128
NKT = K * NT      # 72 ki-chunks of 128
BETA = 1.45
NUM_ROUTING = 3

_CACHE = {}


def _build_nc():
    import concourse.bass as bass
    import concourse.tile as tile
    from concourse import bacc, mybir

    f32 = mybir.dt.float32
    bf16 = mybir.dt.bfloat16
    i32 = mybir.dt.int32
    Alu = mybir.AluOpType
    Act = mybir.ActivationFunctionType
    Ax = mybir.AxisListType

    nc = bacc.Bacc("TRN2", target_bir_lowering=False, debug=False,
                   num_devices=NCORES)

    # host-prepped bf16 inputs (see _prep below); both x layouts come from
    # the host (the extra 1.18MB DMA is cheaper than 72 PE transposes + 72
    # ACT evacuations, which were ~9+18us of it0 engine time)
    xs1 = nc.dram_tensor("xs1", [128, NKT, BL], bf16, kind="ExternalInput").ap()
    xs2 = nc.dram_tensor("xs2", [BL, K * IC], bf16, kind="ExternalInput").ap()
    wk = nc.dram_tensor("wk", [128, NT * JU * K], bf16, kind="ExternalInput").ap()
    out = nc.dram_tensor("out", [BL, JU], f32, kind="ExternalOutput").ap()

    with tile.TileContext(nc) as tc, ExitStack() as ctx:
        consts = ctx.enter_context(tc.tile_pool(name="consts", bufs=1))
        small = ctx.enter_context(tc.tile_pool(name="small", bufs=2))
        scratch = ctx.enter_context(tc.tile_pool(name="scratch", bufs=8))
        psum = ctx.enter_context(tc.tile_pool(name="psum", bufs=1, space="PSUM"))
        dram = ctx.enter_context(tc.tile_pool(name="dram", bufs=1, space="DRAM"))

        # ---- persistent SBUF tensors ----
        x2b = consts.tile([BL, K * IC], bf16)        # x[b, (k i)] (G stationary)
        x1b = consts.tile([128, NKT, BL], bf16)      # x^T per ki-chunk (s stationary)
        w_kju = consts.tile([128, NT, K * JU], bf16)   # W[(i),(k,j,u)]
        wp = consts.tile([128, NT, K * JU], bf16)      # exp-scaled W (iters>0)
        ones = consts.tile([128, 128], bf16)         # Z broadcast matmul lhsT

        # one PSUM tensor = all 8 banks; everything slices into it
        pall = psum.tile([128, K, 512], f32)

        nc.vector.memset(ones, 1.0)

        # ---- ACT exp-table preload: the first real Exp fires right after
        # the it0 exchange lands, on the critical path; load the table now.
        etp = consts.tile([BL, 1], f32)
        nc.vector.memset(etp, 0.0)
        nc.scalar.activation(etp, etp, Act.Exp, scale=1.0)

        # ---- loads on all three DMA rings: x (1.18MB) on the sync HWDGE
        # ring, W (2.95MB) split between the scalar HWDGE ring and the
        # gpsimd SWDGE ring, interleaved so W arrives roughly in t2 order.
        # Only 6 HWDGE dmas total: the Tile HWDGE completion-sem lanes (8)
        # otherwise recycle mid-load and stall pieces for ~10us.
        # s0 is paced by W arrival + the per-dma completion-sem latency,
        # and SWDGE sems are ~+8-9us vs HWDGE's ~2-5us (measured): so W
        # stays entirely on the scalar HWDGE ring (3 pieces, t2-ordered),
        # x1 on sync (7 HWDGE dmas total, inside the 8 sem lanes), and x2
        # rides the gpsimd SWDGE ring where its slow sem is harmless (the
        # G-matmuls only need x2 at ~25us).
        wk_flat = w_kju.rearrange("p t f -> p (t f)")
        for q in range(4):
            qn = NKT // 4
            nc.sync.dma_start(out=x1b[:, q * qn:(q + 1) * qn, :],
                              in_=xs1[:, q * qn:(q + 1) * qn, :])
        nc.gpsimd.dma_start(out=x2b, in_=xs2)
        for c in range(3):
            nc.scalar.dma_start(out=wk_flat[:, c * 3840:(c + 1) * 3840],
                                in_=wk[:, c * 3840:(c + 1) * 3840])

        pbf = pall.bitcast(bf16)                    # [128, K, 1024] bf16 view

        # ---- HAM warm-up: the PE clock-gate runs at 1.2GHz until ~3.4us
        # of sustained activity (and refreezes after ~3.4us idle), which
        # made it0's matmuls run ~2x slow. A chain of junk transposes
        # (serialized on one PSUM slot in columns 640:768, untouched by sp
        # bank0 bf16 0:320 / zp bank1 / G matmuls bf16 0:640) brings the
        # array to 2.4GHz before the real work lands. ----
        def _warm(src2d, n, slot, width=128):
            nJ = min(src2d.shape[1], 128)
            for _ in range(n):
                nc.tensor.transpose(pbf[:nJ, slot, 640:640 + width],
                                    src2d[:, :nJ], ones[:, :width])

        _warm(ones, 16, 5)

        NT_A = 5
        ar_results = {}

        def _fire_ag(it, tag, b_bf, base, nt):
            """Cross-core b sum: AllGather the [128, nt*J] half rank-major,
            read the 8 rank blocks back with two plain DMAs, fold 8->1 on
            DVE. No DMA-transpose anywhere: Tile serializes those against
            all other DMA traffic (incl. collectives), which destroyed the
            half-A/half-B overlap in the transpose-based variant."""
            nJ = nt * J
            bslice = b_bf[:, base:base + nt, :].rearrange("p t j -> p (t j)")
            cc_in = dram.tile([128, nJ], bf16, name=f"ccin{it}{tag}")
            cc_out = dram.tile([NCORES * 128, nJ], bf16,
                               name=f"ccout{it}{tag}", addr_space="Shared")
            eng = nc.scalar if tag == "b" else nc.sync
            eng.dma_start(out=cc_in, in_=bslice)
            nc.gpsimd.collective_compute(
                "AllGather", Alu.bypass,
                replica_groups=[list(range(NCORES))],
                ins=[cc_in[:, :]], outs=[cc_out[:, :]])
            rg = small.tile([128, NCORES, nJ], bf16, name=f"rg{it}{tag}")
            cc_out_v = cc_out.rearrange("(r p) f -> p r f", p=128)
            for h, e2 in ((0, nc.sync), (1, nc.scalar)):
                e2.dma_start(out=rg[:, h * 4:(h + 1) * 4, :],
                             in_=cc_out_v[:, h * 4:(h + 1) * 4, :])
            g4 = small.tile([128, 4, nJ], bf16, name=f"agf4{it}{tag}")
            nc.vector.tensor_add(g4, rg[:, :4], rg[:, 4:])
            g2 = small.tile([128, 2, nJ], bf16, name=f"agf2{it}{tag}")
            nc.vector.tensor_add(g2, g4[:, :2], g4[:, 2:])
            bt = small.tile([128, nt, J], bf16, name=f"bf{it}{tag}")
            nc.vector.tensor_add(bt.rearrange("p t j -> p (t j)"),
                                 g2[:, 0], g2[:, 1])
            ar_results[(it, tag)] = bt
            return bt

        for it in range(NUM_ROUTING):
            # ---- wp = exp(b_sum/B) * w_kju (iters>0). Iteration 0 has
            # uniform c = 1/IC folded into the squash scales, so the matmul
            # rhs is just w_kju directly.
            # Per chunk: ACT fuses exp + (k-half, u) broadcast into a
            # materialized eU4 [128, 4*JU]; the two wp halves are then
            # fully-contiguous bf16 tensor_muls -> DVE 2x mode. The Z
            # ones-matmuls accumulate from a strided eU4 view. ----
            if it > 0:
                bf_parts = [(ar_results[(it - 1, "a")], 0),
                            (ar_results[(it - 1, "b")], NT_A)]

                def bf_of(t2):
                    for tile_, bs in reversed(bf_parts):
                        if t2 >= bs:
                            return tile_[:, t2 - bs, :]
                    raise AssertionError

                eus = []
                for t2 in range(NT):
                    eu = scratch.tile([128, 4, J, U], bf16,
                                      name=f"eu{it}", bufs=3)
                    nc.scalar.activation(
                        eu, bf_of(t2).unsqueeze(1).unsqueeze(-1)
                        .broadcast_to([128, 4, J, U]),
                        Act.Exp, scale=1.0 / B)
                    eus.append(eu)
                    euf = eu.rearrange("p h j u -> p (h j u)")
                    for h in range(2):
                        nc.vector.tensor_mul(
                            wp[:, t2, h * 4 * JU:(h + 1) * 4 * JU],
                            w_kju[:, t2, h * 4 * JU:(h + 1) * 4 * JU],
                            euf)
                rhs_src = wp
            else:
                rhs_src = w_kju

            # ---- s = X1^T @ wp : accumulate 72 chunks into PSUM bank 0.
            # The per-chunk Z matmul (Z[j] = sum_i exp(b[i,j])) rides the
            # PE queue right behind each chunk's s-matmuls; zinv computes
            # on the DVE while the last s-matmuls still stream. ----
            sp = pall[:BL, 0, :JU]
            zinv = None
            for t2 in range(NT):
                for k in range(K):
                    t = k * NT + t2
                    first = (t2 == 0 and k == 0)
                    last = (t2 == NT - 1 and k == K - 1)
                    nc.tensor.matmul(sp, x1b[:, t, :],
                                     rhs_src[:, t2, k * JU:(k + 1) * JU],
                                     start=first, stop=last)
                if it > 0:
                    zp = pall[:, 1, :J]
                    nc.tensor.matmul(zp, ones, eus[t2][:, 0, :, 0],
                                     start=(t2 == 0), stop=(t2 == NT - 1))
                    if t2 == NT - 1:
                        zinv = small.tile([BL, J], f32, name=f"zinv{it}")
                        nc.vector.reciprocal(zinv, zp[:BL, :])

            if it > 0:
                # s_norm = s * (1/Z_j), also evacuates PSUM
                s_sb = small.tile([BL, JU], f32, name=f"s_sb{it}")
                nc.vector.tensor_mul(
                    s_sb.rearrange("b (j u) -> b j u", j=J),
                    sp.rearrange("b (j u) -> b j u", j=J),
                    zinv.unsqueeze(-1).broadcast_to([BL, J, U]))
            else:
                s_sb = small.tile([BL, JU], f32, name=f"s_sb{it}")
                nc.vector.tensor_copy(s_sb, sp)

            # ---- squash (reference quirk: norm over the j axis per (b,u)).
            # All on DVE; sqrt via quake-seed Newton rsqrt (no ACT tables). ----
            ssq = small.tile([BL, JU], f32, name=f"ssq{it}")
            nc.vector.tensor_mul(ssq, s_sb, s_sb)
            msq = small.tile([BL, U], f32, name=f"msq{it}")
            nc.vector.tensor_reduce(
                msq, ssq.rearrange("b (j u) -> b u j", j=J),
                axis=Ax.X, op=Alu.add)
            # iteration 0: s here is actually IC*s, so scale msq by 1/IC^2
            # and fold 1/IC into the final v multiply
            sc2 = 1.0 / (IC * IC) if it == 0 else 1.0
            scv = 1.0 / (IC * IC) if it == 0 else 1.0
            # y ~= rsqrt(msq): quake seed + 1 Newton iteration
            ti = small.tile([BL, U], i32, name=f"ti{it}")
            nc.vector.tensor_scalar(ti, msq.bitcast(i32), 1, 0,
                                    op0=Alu.arith_shift_right,
                                    op1=Alu.logical_shift_left)
            y0i = small.tile([BL, U], i32, name=f"y0i{it}")
            nc.vector.tensor_scalar(y0i, ti, 0x5f3759df, -1,
                                    op0=Alu.subtract, op1=Alu.mult)
            y0 = y0i.bitcast(f32)
            yc = y0
            for n in range(1):
                # y1 = y*(1.5 - 0.5*msq*y^2) in 3 DVE ops: square, then
                # two scalar_tensor_tensor fusions.
                t_a = small.tile([BL, U], f32, name=f"na{it}_{n}")
                nc.vector.tensor_mul(t_a, yc, yc)
                nc.vector.scalar_tensor_tensor(
                    out=t_a, in0=t_a, scalar=-0.5, in1=msq,
                    op0=Alu.mult, op1=Alu.mult)
                t_b = small.tile([BL, U], f32, name=f"nb{it}_{n}")
                nc.vector.scalar_tensor_tensor(
                    out=t_b, in0=t_a, scalar=1.5, in1=yc,
                    op0=Alu.add, op1=Alu.mult)
                yc = t_b
            # f = msq*y * 1/(beta + msq*sc2) (scaled for it0)
            tpb = small.tile([BL, U], f32, name=f"tpb{it}")
            nc.vector.tensor_scalar(tpb, msq, sc2, BETA,
                                    op0=Alu.mult, op1=Alu.add)
            rin = small.tile([BL, U], f32, name=f"rin{it}")
            nc.vector.reciprocal(rin, tpb)
            fv = small.tile([BL, U], f32, name=f"fv{it}")
            nc.vector.tensor_mul(fv, msq, yc)
            nc.vector.tensor_mul(fv, fv, rin)
            # the last iteration needs fp32 v for the output; earlier
            # iterations only feed the bf16 G-matmul rhs, so the final
            # squash op writes bf16 directly (saves the ACT cast + hop).
            vdt = f32 if it == NUM_ROUTING - 1 else bf16
            v = small.tile([BL, JU], vdt, name=f"v{it}")
            nc.vector.scalar_tensor_tensor(
                out=v.rearrange("b (j u) -> b j u", j=J),
                in0=s_sb.rearrange("b (j u) -> b j u", j=J),
                scalar=scv,
                in1=fv.unsqueeze(1).broadcast_to([BL, J, U]),
                op0=Alu.mult, op1=Alu.mult)

            if it == NUM_ROUTING - 1:
                nc.sync.dma_start(out=out[:, :JU // 2], in_=v[:, :JU // 2])
                nc.scalar.dma_start(out=out[:, JU // 2:], in_=v[:, JU // 2:])
                continue
            vb = v

            # ---- G = X2^T-chunks @ v per t2; banks ping-pong in halves
            # (even t2 -> banks 0-3, odd -> 4-7; k packed 2-per-bank).
            # The PSUM bank-linear order IS (k,j,u) -> contiguous ACT
            # evacuation and contiguous DVE product against w_kju.
            # DVE work runs two chunks per instruction (pairs) to amortize
            # instruction overheads: product, 3 fold adds, one u-reduce. ----
            b_part = small.tile([128, NT, J], f32, name=f"bpart{it}")
            b_bf = small.tile([128, NT, J], bf16, name=f"bbf{it}")
            g5 = None

            def _pair_dve(lo, n):
                prod = scratch.tile([128, 2, JU * K], bf16, name="prod",
                                    bufs=2)
                pr = prod[:, :n, :]
                nc.vector.tensor_mul(pr, w_kju[:, lo:lo + n, :],
                                     g5[:, :n, :])
                p3 = pr.rearrange("p c (k f) -> p c k f", k=K)
                f4 = scratch.tile([128, 2, 4, JU], bf16, name="bf4", bufs=2)
                nc.vector.tensor_add(f4[:, :n], p3[:, :, :4], p3[:, :, 4:])
                f2 = scratch.tile([128, 2, 2, JU], bf16, name="bf2", bufs=2)
                nc.vector.tensor_add(f2[:, :n], f4[:, :n, :2], f4[:, :n, 2:])
                f1 = scratch.tile([128, 2, JU], bf16, name="bf1", bufs=2)
                nc.vector.tensor_add(f1[:, :n], f2[:, :n, 0], f2[:, :n, 1])
                nc.vector.tensor_reduce(
                    b_part[:, lo:lo + n, :],
                    f1[:, :n].rearrange("p c (j u) -> p c j u", j=J),
                    axis=Ax.X, op=Alu.add)
                nc.scalar.copy(b_bf[:, lo:lo + n, :], b_part[:, lo:lo + n, :])

            # chunk groups (0,1)(2,3)(4)(5,6)(7,8): half A = 5 chunks fires
            # as early as possible, half B = 4 chunks keeps the LAST
            # exchange flight and it2's trailing wp/s block small.
            for lo, n in ((0, 2), (2, 2), (4, 1), (5, 2), (7, 2)):
                g5 = scratch.tile([128, 2, JU * K], bf16, name="g5", bufs=2)
                for c in range(n):
                    t2 = lo + c
                    b0 = 0 if t2 % 2 == 0 else 4
                    for k in range(K):
                        bank = b0 + k // 2
                        kk = k % 2
                        nc.tensor.matmul(
                            pall[:, bank, kk * JU:(kk + 1) * JU],
                            x2b[:, (k * IC + t2 * 128):
                                (k * IC + t2 * 128) + 128],
                            vb, start=True, stop=True)
                    nc.scalar.copy(
                        g5[:, c, :].rearrange("p (b f) -> p b f", b=4),
                        pall[:, b0:b0 + 4, :2 * JU])
                _pair_dve(lo, n)
                if lo + n == NT_A:
                    # half A's doorbell fires mid b-update (for it0, while
                    # the ncfw barrier is still settling); its flight
                    # overlaps the B half, and the next iteration's half-A
                    # exp/wp/s overlaps half B's flight.
                    _fire_ag(it, "a", b_bf, 0, NT_A)
            _fire_ag(it, "b", b_bf, NT_A, NT - NT_A)
            bt = ar_results[(it, "a")]
            # re-warm the PE while the exchange result is consumed by the
            # DVE/ACT ramp (folds, exp, first wp chunks): these junk
            # transposes are gated on the result tile, so they execute
            # exactly in that window, right before the next s-matmuls.
            # ~12 full-width ops ~= the 3.4us HAM activity threshold
            # (narrower/fewer ops measurably under-warm the array).
            _warm(bt.rearrange("p t j -> p (t j)"), 12, 6)

    nc.compile()
    return nc


def _prep(x, W):
    """Host-side prep: bf16 cast + device layouts for x and W."""
    import ml_dtypes

    bf16 = ml_dtypes.bfloat16
    x = np.asarray(x, dtype=np.float32)
    W = np.asarray(W, dtype=np.float32)
    xb = x.astype(bf16)                      # (B, K, IC)
    # W (k,j,u): [p, (t2, k j u)]
    wk = np.ascontiguousarray(
        W.reshape(NT, 128, J, U, K).transpose(1, 0, 4, 2, 3)
        .reshape(128, NT * K * J * U).astype(bf16))
    in_maps = []
    for c in range(NCORES):
        rows = xb[c * BL:(c + 1) * BL]       # (BL, K, IC)
        xs1 = np.ascontiguousarray(
            rows.reshape(BL, K, NT, 128).transpose(3, 1, 2, 0)
            .reshape(128, NKT, BL))
        xs2 = np.ascontiguousarray(rows.reshape(BL, K * IC))
        in_maps.append({
            "xs1": xs1,
            "xs2": xs2,
            "wk": wk,
        })
    return in_maps


def _run(x, W, trace=False, **kw):
    from concourse import bass_utils

    nc = _get_nc()
    in_maps = _prep(x, W)
    res = bass_utils.run_bass_kernel_spmd(
        nc, in_maps, core_ids=list(range(NCORES)), trace=trace, **kw)
    outs = [res.results[c]["out"] for c in range(NCORES)]
    full = np.concatenate(outs, axis=0).reshape(B, J, 4, U // 4)
    return full, res


def _get_nc():
    if "nc" not in _CACHE:
        _CACHE["nc"] = _build_nc()
    return _CACHE["nc"]


def kernel(x, W):
    full, _ = _run(x, W, trace=False)
    return full
